# revision 1
# baseline (speedup 1.0000x reference)
"""Complex-magnitude MaxPool2d (k=2, s=2) Trainium2 Bass kernel.

Input  x:  [16, 2, 64, 224, 224] f32  (plane 0 = real, plane 1 = imag)
Output:    [16, 2, 64, 112, 112] f32  (value of the window element with the
                                       largest |z|^2 = re^2 + im^2)

Sharding: pure data parallel over batch: 16 / 8 cores = 2 examples per core.
Per core the 2(batch) x 64(channel) = 128 image planes map 1:1 onto the 128
SBUF partitions; DMA moves 28 image rows at a time in a single 128-partition
dma_start (one transfer spans all 16 SBUF AXI ports and amortizes the ~2us
per-dma fixed cost); compute runs on 14-row subchunks.

Selection reproduces jnp.argmax's first-index tie-break exactly:
horizontal pass first (left/even column wins ties via is_ge), then vertical
(top row wins ties).  norm2 = fl(fl(re*re)+fl(im*im)) in f32 — ACT's Square
activation and GPSIMD's f32 add are bit-exact with the reference expression
(hardware-verified), so selections match the reference everywhere,
including exact ties.

Engine split (measured rates):
  ScalarE : squares (one ACT op per subchunk), select pre-fill copies
  GPSIMD  : norm add (in place over the squares)
  VectorE : is_ge masks + copy_predicated selects.  Masks and predicated
            dst stay contiguous (2x faster than strided), and each pred
            selects re+im together via a step-0 broadcast mask.
  DMA     : 128-partition transfers; outputs staged to long runs.
"""

import numpy as np

import concourse.bass as bass
import concourse.mybir as mybir
from concourse import bacc, bass_utils, tile

# Per-core shard geometry (hardcoded; kernel.py must be self-contained).
NCORES = 8
B = 2            # batch per core
RI = 2           # real/imag planes
C = 64           # channels
H = W = 224
HO, WO = H // 2, W // 2
P = 128          # SBUF partitions = B * C
RD = 28          # image rows per DMA chunk
R = 14           # image rows per compute subchunk
SUB = RD // R    # compute subchunks per DMA chunk (2)
NCHUNK = H // RD  # 8
N = R * W        # free elements per plane per subchunk (3136)
GROUP = 4        # subchunks staged per output store (28 output rows)
SROWS = GROUP * (R // 2)

F32 = mybir.dt.float32
I8 = mybir.dt.uint8
OP = mybir.AluOpType
ACTF = mybir.ActivationFunctionType

_NC_CACHE = []


def _build_nc() -> bass.Bass:
    nc = bacc.Bacc("TRN2", target_bir_lowering=False, debug=False)
    # host pre-transposed: partition-major [b*c, ri, H, W] so every DMA is a
    # single-dim 128-partition transfer (hits all 16 SBUF AXI ports)
    x = nc.dram_tensor("x", [P, RI, H, W], F32, kind="ExternalInput").ap()
    out = nc.dram_tensor("out", [P, RI, HO, WO], F32, kind="ExternalOutput").ap()

    with tile.TileContext(nc) as tc:
        with tc.tile_pool(name="pool", bufs=2) as pool:
            stage = None
            subidx = 0
            for k in range(NCHUNK):
                r0 = k * RD
                # xri free layout per partition: [ri][row 0..RD)[col]
                xri = pool.tile([P, RI * RD * W], F32, tag="xri")
                nrw = RD * W
                nc.sync.dma_start(
                    out=xri.rearrange("p (ri f) -> p ri f", ri=RI),
                    in_=x[:, :, r0 : r0 + RD, :].rearrange("p ri r w -> p ri (r w)"),
                )

                for s in range(SUB):
                    # subchunk views: rows rs..rs+R of each plane
                    xri6 = xri.rearrange(
                        "p (ri r w t) -> p ri r w t", ri=RI, r=RD, w=WO, t=2
                    )[:, :, s * R : (s + 1) * R, :, :]

                    # squares of re+im rows in one ACT op; norm2 in place
                    # over the re half; im half is reused as riH below
                    sqri = pool.tile([P, RI * N], F32, tag="sqri")
                    nc.scalar.activation(
                        out=sqri.rearrange(
                            "p (ri r w t) -> p ri r w t", ri=RI, r=R, w=WO, t=2
                        ),
                        in_=xri6,
                        func=ACTF.Square,
                    )
                    # norm add on DVE: GPSIMD shares a SBUF read port with
                    # DVE 2-stream ops and stalls them 2x when overlapped,
                    # so keeping GPSIMD idle is a net win
                    nrm = sqri[:, :N]
                    nc.vector.tensor_tensor(
                        out=nrm, in0=nrm, in1=sqri[:, N:], op=OP.add
                    )

                    nrm4 = nrm.rearrange("p (r w t) -> p r w t", r=R, w=WO, t=2)
                    nE, nO = nrm4[:, :, :, 0], nrm4[:, :, :, 1]

                    # horizontal mask (contiguous u8): even/left wins ties
                    cH = pool.tile([P, R * WO], I8, tag="cH")
                    cH3 = cH.rearrange("p (r w) -> p r w", r=R, w=WO)
                    nc.vector.tensor_tensor(out=cH3, in0=nE, in1=nO, op=OP.is_ge)
                    # horizontal norm max -> nrm odd slots (in place)
                    nc.vector.tensor_tensor(out=nO, in0=nE, in1=nO, op=OP.max)

                    # horizontal select of (re, im) together into the dead
                    # im-squares half: pre-fill with odd/right, overwrite
                    # where cH
                    riH = sqri[:, N:]
                    riH4 = riH.rearrange("p (ri r w) -> p ri r w", ri=RI, r=R, w=WO)
                    nc.scalar.copy(out=riH4, in_=xri6[:, :, :, :, 1])
                    cHb = cH3.unsqueeze(1).broadcast_to([P, RI, R, WO])
                    nc.vector.copy_predicated(
                        out=riH4, mask=cHb, data=xri6[:, :, :, :, 0]
                    )

                    # vertical mask from the horizontal maxes: top wins ties
                    nrm5 = nrm.rearrange(
                        "p (rp rt w t) -> p rp rt w t", rp=R // 2, rt=2, w=WO, t=2
                    )
                    cV = pool.tile([P, (R // 2) * WO], I8, tag="cV")
                    cV3 = cV.rearrange("p (rp w) -> p rp w", rp=R // 2, w=WO)
                    nc.vector.tensor_tensor(
                        out=cV3,
                        in0=nrm5[:, :, 0, :, 1],
                        in1=nrm5[:, :, 1, :, 1],
                        op=OP.is_ge,
                    )

                    # vertical select into the staged output tile
                    riH5 = riH.rearrange(
                        "p (ri rp rt w) -> p ri rp rt w",
                        ri=RI, rp=R // 2, rt=2, w=WO,
                    )
                    if subidx % GROUP == 0:
                        stage = pool.tile([P, RI * SROWS * WO], F32, tag="stage")
                    stage4 = stage.rearrange(
                        "p (ri r w) -> p ri r w", ri=RI, r=SROWS, w=WO
                    )
                    s0 = (subidx % GROUP) * (R // 2)
                    dst = stage4[:, :, s0 : s0 + R // 2, :]
                    nc.scalar.copy(out=dst, in_=riH5[:, :, :, 1, :])
                    cVb = cV3.unsqueeze(1).broadcast_to([P, RI, R // 2, WO])
                    nc.vector.copy_predicated(
                        out=dst, mask=cVb, data=riH5[:, :, :, 0, :]
                    )

                    if (subidx + 1) % GROUP == 0:
                        g0 = (subidx + 1 - GROUP) * (R // 2)
                        nc.sync.dma_start(
                            out=out[:, :, g0 : g0 + SROWS, :].rearrange(
                                "p ri r w -> p ri (r w)"
                            ),
                            in_=stage.rearrange("p (ri f) -> p ri f", ri=RI),
                        )
                    subidx += 1
    nc.compile()
    return nc


def get_nc() -> bass.Bass:
    if not _NC_CACHE:
        _NC_CACHE.append(_build_nc())
    return _NC_CACHE[0]


def kernel(x: np.ndarray, **run_kwargs) -> np.ndarray:
    nc = get_nc()
    xs = np.asarray(x, dtype=np.float32)
    assert xs.shape == (NCORES * B, RI, C, H, W), xs.shape
    # [16,2,64,H,W] -> per core [b,c,ri,H,W] flattened to [128,ri,H,W]
    xt = np.ascontiguousarray(xs.transpose(0, 2, 1, 3, 4))
    in_maps = [
        {"x": xt[B * i : B * (i + 1)].reshape(P, RI, H, W)} for i in range(NCORES)
    ]
    res = bass_utils.run_bass_kernel_spmd(
        nc, in_maps, core_ids=list(range(NCORES)), **run_kwargs
    )
    # per-core [128,ri,HO,WO] -> [b,c,ri,HO,WO] -> [b,ri,c,HO,WO]
    out = np.concatenate(
        [
            res.results[i]["out"].reshape(B, C, RI, HO, WO).transpose(0, 2, 1, 3, 4)
            for i in range(NCORES)
        ],
        axis=0,
    )
    if run_kwargs:
        kernel.last_results = res
    return np.ascontiguousarray(out)



# revision 3
# speedup vs baseline: 1.1137x; 1.1137x over previous
"""Complex-magnitude MaxPool2d (k=2, s=2) Trainium2 Bass kernel.

Input  x:  [16, 2, 64, 224, 224] f32  (plane 0 = real, plane 1 = imag)
Output:    [16, 2, 64, 112, 112] f32  (value of the window element with the
                                       largest |z|^2 = re^2 + im^2)

Sharding: pure data parallel over batch: 16 / 8 cores = 2 examples per core;
2(batch) x 64(channel) = 128 image planes map 1:1 onto SBUF partitions.

Host pre-interleaves to [p, h, (w2 t ri)] so each chunk's DMA is one
contiguous 25KB-per-partition descriptor and every select handles a
window's (re, im) together via a step-0 broadcast mask.

  ACT  : squares (one contiguous Square per chunk), select pre-fills
  DVE  : norm add, is_ge masks, horizontal max, copy_predicated selects
  DMA  : 16 input chunks (14 rows), 16 output stores (7 rows)

Selection reproduces jnp.argmax's first-index tie-break (horizontal
is_ge: even/left wins; vertical is_ge: top wins); norm arithmetic is
fl(fl(re^2)+fl(im^2)), bit-exact with the reference.
"""

import numpy as np

import concourse.bass as bass
import concourse.mybir as mybir
from concourse import bacc, bass_utils, tile

NCORES = 8
B = 2            # batch per core
RI = 2           # real/imag
C = 64           # channels
H = W = 224
HO, WO = H // 2, W // 2
P = 128          # SBUF partitions = B * C
R = 14           # image rows per chunk (one DMA = one compute step)
NCHUNK = H // R  # 16
W2 = W // 2      # 112 column pairs
RP = R // 2      # 7 output rows per chunk
N = R * W        # values per partition per chunk per plane... (R*W2*2 norms)

F32 = mybir.dt.float32
U8 = mybir.dt.uint8
OP = mybir.AluOpType
ACTF = mybir.ActivationFunctionType

_NC_CACHE = []


def _build_nc() -> bass.Bass:
    nc = bacc.Bacc("TRN2", target_bir_lowering=False, debug=False)
    x = nc.dram_tensor("x", [P, H, W * RI], F32, kind="ExternalInput").ap()
    out = nc.dram_tensor("out", [P, HO, WO * RI], F32, kind="ExternalOutput").ap()

    NV = R * W2 * 2          # norms per chunk (3136)
    NH = R * W2              # horizontal windows per chunk (1568)
    NO = RP * W2             # output windows per chunk (784)

    with tile.TileContext(nc) as tc:
        with tc.tile_pool(name="pool", bufs=2) as pool:
            for k in range(NCHUNK):
                xri = pool.tile([P, R * W * RI], F32, tag="xri")
                nc.sync.dma_start(
                    out=xri.rearrange("p (r f) -> p r f", r=R),
                    in_=x[:, k * R : (k + 1) * R, :],
                )

                # squares of the whole chunk in one contiguous ACT op
                sq = pool.tile([P, R * W * RI], F32, tag="sq")
                nc.scalar.activation(out=sq, in_=xri, func=ACTF.Square)

                # norm2 = re^2 + im^2 (both stride-2 reads, contiguous out)
                nrm = pool.tile([P, NV], F32, tag="nrm")
                sqp = sq.rearrange("p (n ri) -> p n ri", ri=RI)
                nc.vector.tensor_tensor(
                    out=nrm, in0=sqp[:, :, 0], in1=sqp[:, :, 1], op=OP.add
                )

                nrm2 = nrm.rearrange("p (n t) -> p n t", t=2)
                nE, nO = nrm2[:, :, 0], nrm2[:, :, 1]

                # horizontal mask (even/left wins ties) + horizontal max
                cH = pool.tile([P, NH], U8, tag="cH")
                nc.vector.tensor_tensor(out=cH, in0=nE, in1=nO, op=OP.is_ge)
                mH = pool.tile([P, NH], F32, tag="mH")
                nc.vector.tensor_tensor(out=mH, in0=nE, in1=nO, op=OP.max)

                # horizontal select of (re,im) pairs: pre-fill odd candidates,
                # overwrite where even wins (mask broadcast over the pair)
                xp = xri.rearrange("p (n t ri) -> p n t ri", t=2, ri=RI)
                riH = pool.tile([P, NH * RI], F32, tag="riH")
                riH3 = riH.rearrange("p (n ri) -> p n ri", ri=RI)
                nc.scalar.copy(out=riH3, in_=xp[:, :, 1, :])
                cHb = cH.unsqueeze(2).broadcast_to([P, NH, RI])
                nc.vector.copy_predicated(
                    out=riH3, mask=cHb, data=xp[:, :, 0, :]
                )

                # vertical mask from horizontal maxes (top wins ties)
                mHr = mH.rearrange("p (rp rt w2) -> p rp rt w2", rp=RP, rt=2, w2=W2)
                cV = pool.tile([P, NO], U8, tag="cV")
                cV3 = cV.rearrange("p (rp w2) -> p rp w2", rp=RP, w2=W2)
                nc.vector.tensor_tensor(
                    out=cV3, in0=mHr[:, :, 0, :], in1=mHr[:, :, 1, :], op=OP.is_ge
                )

                # vertical select into the output tile
                riHr = riH.rearrange(
                    "p (rp rt w2 ri) -> p rp rt w2 ri", rp=RP, rt=2, w2=W2, ri=RI
                )
                outT = pool.tile([P, NO * RI], F32, tag="outT")
                outT4 = outT.rearrange(
                    "p (rp w2 ri) -> p rp w2 ri", rp=RP, w2=W2, ri=RI
                )
                nc.scalar.copy(out=outT4, in_=riHr[:, :, 1, :, :])
                cVb = cV3.unsqueeze(3).broadcast_to([P, RP, W2, RI])
                nc.vector.copy_predicated(
                    out=outT4, mask=cVb, data=riHr[:, :, 0, :, :]
                )

                nc.sync.dma_start(
                    out=out[:, k * RP : (k + 1) * RP, :],
                    in_=outT.rearrange("p (rp f) -> p rp f", rp=RP),
                )
    nc.compile()
    return nc


def get_nc() -> bass.Bass:
    if not _NC_CACHE:
        _NC_CACHE.append(_build_nc())
    return _NC_CACHE[0]


def kernel(x: np.ndarray, **run_kwargs) -> np.ndarray:
    nc = get_nc()
    xs = np.asarray(x, dtype=np.float32)
    assert xs.shape == (NCORES * B, RI, C, H, W), xs.shape
    # [16,2,64,H,W] -> [16,64,H,W,2] (ri innermost) -> per-core [128,H,W*2]
    xt = np.ascontiguousarray(xs.transpose(0, 2, 3, 4, 1))
    in_maps = [
        {"x": xt[B * i : B * (i + 1)].reshape(P, H, W * RI)} for i in range(NCORES)
    ]
    res = bass_utils.run_bass_kernel_spmd(
        nc, in_maps, core_ids=list(range(NCORES)), **run_kwargs
    )
    # per-core [128, HO, WO*2] -> [b, c, HO, WO, ri] -> [b, ri, c, HO, WO]
    outs = [
        res.results[i]["out"].reshape(B, C, HO, WO, RI).transpose(0, 4, 1, 2, 3)
        for i in range(NCORES)
    ]
    out = np.concatenate(outs, axis=0)
    if run_kwargs:
        kernel.last_results = res
    return np.ascontiguousarray(out)


# revision 4
# speedup vs baseline: 1.2459x; 1.1187x over previous
"""Complex-magnitude MaxPool2d (k=2, s=2) Trainium2 Bass kernel.

Input  x:  [16, 2, 64, 224, 224] f32  (plane 0 = real, plane 1 = imag)
Output:    [16, 2, 64, 112, 112] f32  (value of the window element with the
                                       largest |z|^2 = re^2 + im^2)

Sharding: pure data parallel over batch: 16 / 8 cores = 2 examples per core;
2(batch) x 64(channel) = 128 image planes map 1:1 onto SBUF partitions.

Host layout per row: [even-column (re,im) pairs | odd-column pairs]
([p, h, t, w2, ri]).  One contiguous 25KB-per-partition DMA per 14-row
chunk, and every heavy engine stream (masks, maxes, select data and
pre-fills) is contiguous; only the norm add reads stride-2 and the
copy_predicated masks broadcast step-0 over the (re,im) pair.

  ACT  : squares (one contiguous Square per chunk), select pre-fills
  DVE  : norm add, is_ge masks, horizontal max, copy_predicated selects
  DMA  : 16 input chunks (14 rows), 16 output stores (7 rows), input
         tile triple-buffered so the 9us chunk DMA stays 2 chunks ahead

Selection reproduces jnp.argmax's first-index tie-break (horizontal
is_ge: even/left wins; vertical is_ge: top wins); norm arithmetic is
fl(fl(re^2)+fl(im^2)), bit-exact with the reference.
"""

import numpy as np

import concourse.bass as bass
import concourse.mybir as mybir
from concourse import bacc, bass_utils, tile

NCORES = 8
B = 2            # batch per core
RI = 2           # real/imag
C = 64           # channels
H = W = 224
HO, WO = H // 2, W // 2
P = 128          # SBUF partitions = B * C
R = 14           # image rows per chunk (one DMA = one compute step)
NCHUNK = H // R  # 16
W2 = W // 2      # 112 column pairs
RP = R // 2      # 7 output rows per chunk

F32 = mybir.dt.float32
U8 = mybir.dt.uint8
OP = mybir.AluOpType
ACTF = mybir.ActivationFunctionType

_NC_CACHE = []


def _build_nc() -> bass.Bass:
    nc = bacc.Bacc("TRN2", target_bir_lowering=False, debug=False)
    x = nc.dram_tensor("x", [P, H, W * RI], F32, kind="ExternalInput").ap()
    out = nc.dram_tensor("out", [P, HO, WO * RI], F32, kind="ExternalOutput").ap()

    NH = R * W2              # horizontal windows per chunk (1568)
    NO = RP * W2             # output windows per chunk (784)
    NVAL = R * W * RI        # f32 values per chunk (6272)

    with tile.TileContext(nc) as tc:
        with tc.tile_pool(name="pool", bufs=2) as pool:
            for k in range(NCHUNK):
                xri = pool.tile([P, NVAL], F32, tag="xri", bufs=3)
                nc.sync.dma_start(
                    out=xri.rearrange("p (r f) -> p r f", r=R),
                    in_=x[:, k * R : (k + 1) * R, :],
                )

                # squares of the whole chunk in one contiguous ACT op
                sq = pool.tile([P, NVAL], F32, tag="sq")
                nc.scalar.activation(out=sq, in_=xri, func=ACTF.Square)

                # norm2 = re^2 + im^2 (stride-2 reads, contiguous out; the
                # row-block layout keeps even norms in [:, :NH], odd in [NH:])
                nrm = pool.tile([P, NH * 2], F32, tag="nrm")
                sqp = sq.rearrange("p (n ri) -> p n ri", ri=RI)
                nc.vector.tensor_tensor(
                    out=nrm, in0=sqp[:, :, 0], in1=sqp[:, :, 1], op=OP.add
                )
                nE3 = nrm.rearrange("p (r t w2) -> p r t w2", r=R, t=2, w2=W2)
                nE, nO = nE3[:, :, 0, :], nE3[:, :, 1, :]

                # horizontal mask (even/left wins ties) + horizontal max
                cH = pool.tile([P, NH], U8, tag="cH")
                cH3 = cH.rearrange("p (r w2) -> p r w2", r=R, w2=W2)
                nc.vector.tensor_tensor(out=cH3, in0=nE, in1=nO, op=OP.is_ge)
                mH = pool.tile([P, NH], F32, tag="mH")
                mH3 = mH.rearrange("p (r w2) -> p r w2", r=R, w2=W2)
                nc.vector.tensor_tensor(out=mH3, in0=nE, in1=nO, op=OP.max)

                # horizontal select of (re,im) pairs: pre-fill odd-column
                # candidates (contiguous block), overwrite where even wins
                xp = xri.rearrange(
                    "p (r t w2 ri) -> p r t w2 ri", r=R, t=2, w2=W2, ri=RI
                )
                riH = pool.tile([P, NH * RI], F32, tag="riH")
                riH4 = riH.rearrange(
                    "p (r w2 ri) -> p r w2 ri", r=R, w2=W2, ri=RI
                )
                nc.scalar.copy(out=riH4, in_=xp[:, :, 1, :, :])
                cHb = cH3.unsqueeze(3).broadcast_to([P, R, W2, RI])
                nc.vector.copy_predicated(
                    out=riH4, mask=cHb, data=xp[:, :, 0, :, :]
                )

                # vertical mask from horizontal maxes (top wins ties)
                mHr = mH.rearrange("p (rp rt w2) -> p rp rt w2", rp=RP, rt=2, w2=W2)
                cV = pool.tile([P, NO], U8, tag="cV")
                cV3 = cV.rearrange("p (rp w2) -> p rp w2", rp=RP, w2=W2)
                nc.vector.tensor_tensor(
                    out=cV3, in0=mHr[:, :, 0, :], in1=mHr[:, :, 1, :], op=OP.is_ge
                )

                # vertical select into the output tile
                riHr = riH.rearrange(
                    "p (rp rt w2 ri) -> p rp rt w2 ri", rp=RP, rt=2, w2=W2, ri=RI
                )
                outT = pool.tile([P, NO * RI], F32, tag="outT")
                outT4 = outT.rearrange(
                    "p (rp w2 ri) -> p rp w2 ri", rp=RP, w2=W2, ri=RI
                )
                nc.scalar.copy(out=outT4, in_=riHr[:, :, 1, :, :])
                cVb = cV3.unsqueeze(3).broadcast_to([P, RP, W2, RI])
                nc.vector.copy_predicated(
                    out=outT4, mask=cVb, data=riHr[:, :, 0, :, :]
                )

                nc.sync.dma_start(
                    out=out[:, k * RP : (k + 1) * RP, :],
                    in_=outT.rearrange("p (rp f) -> p rp f", rp=RP),
                )
    nc.compile()
    return nc


def get_nc() -> bass.Bass:
    if not _NC_CACHE:
        _NC_CACHE.append(_build_nc())
    return _NC_CACHE[0]


def kernel(x: np.ndarray, **run_kwargs) -> np.ndarray:
    nc = get_nc()
    xs = np.asarray(x, dtype=np.float32)
    assert xs.shape == (NCORES * B, RI, C, H, W), xs.shape
    # [16,2,64,H,W] -> [16,64,H,W2,t,2] -> row blocks [16,64,H,t,W2,2]
    xt = xs.transpose(0, 2, 3, 4, 1).reshape(NCORES * B, C, H, W2, 2, RI)
    xt = np.ascontiguousarray(xt.transpose(0, 1, 2, 4, 3, 5))
    in_maps = [
        {"x": xt[B * i : B * (i + 1)].reshape(P, H, W * RI)} for i in range(NCORES)
    ]
    res = bass_utils.run_bass_kernel_spmd(
        nc, in_maps, core_ids=list(range(NCORES)), **run_kwargs
    )
    # per-core [128, HO, WO*2] -> [b, c, HO, WO, ri] -> [b, ri, c, HO, WO]
    outs = [
        res.results[i]["out"].reshape(B, C, HO, WO, RI).transpose(0, 4, 1, 2, 3)
        for i in range(NCORES)
    ]
    out = np.concatenate(outs, axis=0)
    if run_kwargs:
        kernel.last_results = res
    return np.ascontiguousarray(out)


# revision 6
# speedup vs baseline: 1.2528x; 1.0056x over previous
"""Complex-magnitude MaxPool2d (k=2, s=2) Trainium2 Bass kernel.

Input  x:  [16, 2, 64, 224, 224] f32  (plane 0 = real, plane 1 = imag)
Output:    [16, 2, 64, 112, 112] f32  (value of the window element with the
                                       largest |z|^2 = re^2 + im^2)

Sharding: pure data parallel over batch: 16 / 8 cores = 2 examples per core;
2(batch) x 64(channel) = 128 image planes map 1:1 onto SBUF partitions.

Host layout per row: [even-column (re,im) pairs | odd-column pairs]
([p, h, t, w2, ri]).  One contiguous 25KB-per-partition DMA per 14-row
chunk, and every heavy engine stream (masks, maxes, select data and
pre-fills) is contiguous; only the norm add reads stride-2 and the
copy_predicated masks broadcast step-0 over the (re,im) pair.

  ACT  : squares (one contiguous Square per chunk), select pre-fills
  DVE  : norm add, is_ge masks, horizontal max, copy_predicated selects
  DMA  : 16 input chunks (14 rows), 16 output stores (7 rows), input
         tile triple-buffered so the 9us chunk DMA stays 2 chunks ahead

Selection reproduces jnp.argmax's first-index tie-break (horizontal
is_ge: even/left wins; vertical is_ge: top wins); norm arithmetic is
fl(fl(re^2)+fl(im^2)), bit-exact with the reference.
"""

import numpy as np

import concourse.bass as bass
import concourse.mybir as mybir
from concourse import bacc, bass_utils, tile

NCORES = 8
B = 2            # batch per core
RI = 2           # real/imag
C = 64           # channels
H = W = 224
HO, WO = H // 2, W // 2
P = 128          # SBUF partitions = B * C
R = 14           # image rows per chunk (one DMA = one compute step)
NCHUNK = H // R  # 16
W2 = W // 2      # 112 column pairs
RP = R // 2      # 7 output rows per chunk

F32 = mybir.dt.float32
BF16 = mybir.dt.bfloat16
U8 = mybir.dt.uint8
OP = mybir.AluOpType
ACTF = mybir.ActivationFunctionType

# chunk row schedule: tiny leading chunks collapse the pipeline-fill ramp
# (a full 14-row chunk needs ~9us DMA + 5.5us of squares before the vector
# engine can start); sizes must be even and sum to H
CHUNKS = [2, 4, 8] + [14] * 15
assert sum(CHUNKS) == H

_NC_CACHE = []


def _build_nc() -> bass.Bass:
    nc = bacc.Bacc("TRN2", target_bir_lowering=False, debug=False)
    x = nc.dram_tensor("x", [P, H, W * RI], F32, kind="ExternalInput").ap()
    out = nc.dram_tensor("out", [P, HO, WO * RI], F32, kind="ExternalOutput").ap()

    with tile.TileContext(nc) as tc:
        with tc.tile_pool(name="pool", bufs=2) as pool:
            r0 = 0
            for k, Rk in enumerate(CHUNKS):
                RPk = Rk // 2
                NH = Rk * W2             # horizontal windows per chunk
                NO = RPk * W2            # output windows per chunk
                NVAL = Rk * W * RI       # f32 values per chunk

                xri = pool.tile([P, R * W * RI], F32, tag="xri", name="xri", bufs=3)[
                    :, :NVAL
                ]
                nc.sync.dma_start(
                    out=xri.rearrange("p (r f) -> p r f", r=Rk),
                    in_=x[:, r0 : r0 + Rk, :],
                )

                # squares of the whole chunk in one contiguous ACT op
                sq = pool.tile([P, R * W * RI], F32, tag="sq", name="sq")[:, :NVAL]
                nc.scalar.activation(out=sq, in_=xri, func=ACTF.Square)

                # norm2 = re^2 + im^2 (stride-2 reads, contiguous out)
                nrm = pool.tile([P, R * W2 * 2], F32, tag="nrm", name="nrm")[:, : NH * 2]
                sqp = sq.rearrange("p (n ri) -> p n ri", ri=RI)
                nc.vector.tensor_tensor(
                    out=nrm, in0=sqp[:, :, 0], in1=sqp[:, :, 1], op=OP.add
                )
                nE3 = nrm.rearrange("p (r t w2) -> p r t w2", r=Rk, t=2, w2=W2)
                nE, nO = nE3[:, :, 0, :], nE3[:, :, 1, :]

                # horizontal mask (even/left wins ties) + horizontal max.
                # masks are bf16 0/1.0: bitcast to u8 gives bytes (0x80,0x3f)
                # per true lane -- a pair-broadcast mask with no step-0 AP
                cH = pool.tile([P, R * W2], BF16, tag="cH", name="cH")[:, :NH]
                cH3 = cH.rearrange("p (r w2) -> p r w2", r=Rk, w2=W2)
                nc.vector.tensor_tensor(out=cH3, in0=nE, in1=nO, op=OP.is_ge)
                mH = pool.tile([P, R * W2], F32, tag="mH", name="mH")[:, :NH]
                mH3 = mH.rearrange("p (r w2) -> p r w2", r=Rk, w2=W2)
                nc.vector.tensor_tensor(out=mH3, in0=nE, in1=nO, op=OP.max)

                # horizontal select of (re,im) pairs: pre-fill odd-column
                # candidates (contiguous block), overwrite where even wins
                xp = xri.rearrange(
                    "p (r t w2 ri) -> p r t w2 ri", r=Rk, t=2, w2=W2, ri=RI
                )
                riH = pool.tile([P, R * W2 * RI], F32, tag="riH", name="riH")[:, : NH * RI]
                riH4 = riH.rearrange(
                    "p (r w2 ri) -> p r w2 ri", r=Rk, w2=W2, ri=RI
                )
                nc.scalar.copy(out=riH4, in_=xp[:, :, 1, :, :])
                nc.vector.copy_predicated(
                    out=riH4,
                    mask=cH.bitcast(U8).rearrange(
                        "p (r w2 ri) -> p r w2 ri", r=Rk, w2=W2, ri=RI
                    ),
                    data=xp[:, :, 0, :, :],
                )

                # vertical mask from horizontal maxes (top wins ties)
                mHr = mH.rearrange(
                    "p (rp rt w2) -> p rp rt w2", rp=RPk, rt=2, w2=W2
                )
                cV = pool.tile([P, RP * W2], BF16, tag="cV", name="cV")[:, :NO]
                cV3 = cV.rearrange("p (rp w2) -> p rp w2", rp=RPk, w2=W2)
                nc.vector.tensor_tensor(
                    out=cV3, in0=mHr[:, :, 0, :], in1=mHr[:, :, 1, :], op=OP.is_ge
                )

                # vertical select into the output tile
                riHr = riH.rearrange(
                    "p (rp rt w2 ri) -> p rp rt w2 ri", rp=RPk, rt=2, w2=W2, ri=RI
                )
                outT = pool.tile([P, RP * W2 * RI], F32, tag="outT", name="outT")[:, : NO * RI]
                outT4 = outT.rearrange(
                    "p (rp w2 ri) -> p rp w2 ri", rp=RPk, w2=W2, ri=RI
                )
                nc.scalar.copy(out=outT4, in_=riHr[:, :, 1, :, :])
                nc.vector.copy_predicated(
                    out=outT4,
                    mask=cV.bitcast(U8).rearrange(
                        "p (rp w2 ri) -> p rp w2 ri", rp=RPk, w2=W2, ri=RI
                    ),
                    data=riHr[:, :, 0, :, :],
                )

                nc.sync.dma_start(
                    out=out[:, r0 // 2 : r0 // 2 + RPk, :],
                    in_=outT.rearrange("p (rp f) -> p rp f", rp=RPk),
                )
                r0 += Rk
    nc.compile()
    return nc


def get_nc() -> bass.Bass:
    if not _NC_CACHE:
        _NC_CACHE.append(_build_nc())
    return _NC_CACHE[0]


def kernel(x: np.ndarray, **run_kwargs) -> np.ndarray:
    nc = get_nc()
    xs = np.asarray(x, dtype=np.float32)
    assert xs.shape == (NCORES * B, RI, C, H, W), xs.shape
    # [16,2,64,H,W] -> [16,64,H,W2,t,2] -> row blocks [16,64,H,t,W2,2]
    xt = xs.transpose(0, 2, 3, 4, 1).reshape(NCORES * B, C, H, W2, 2, RI)
    xt = np.ascontiguousarray(xt.transpose(0, 1, 2, 4, 3, 5))
    in_maps = [
        {"x": xt[B * i : B * (i + 1)].reshape(P, H, W * RI)} for i in range(NCORES)
    ]
    res = bass_utils.run_bass_kernel_spmd(
        nc, in_maps, core_ids=list(range(NCORES)), **run_kwargs
    )
    # per-core [128, HO, WO*2] -> [b, c, HO, WO, ri] -> [b, ri, c, HO, WO]
    outs = [
        res.results[i]["out"].reshape(B, C, HO, WO, RI).transpose(0, 4, 1, 2, 3)
        for i in range(NCORES)
    ]
    out = np.concatenate(outs, axis=0)
    if run_kwargs:
        kernel.last_results = res
    return np.ascontiguousarray(out)


# revision 8
# speedup vs baseline: 1.2585x; 1.0045x over previous
"""Complex-magnitude MaxPool2d (k=2, s=2) Trainium2 Bass kernel.

Input  x:  [16, 2, 64, 224, 224] f32  (plane 0 = real, plane 1 = imag)
Output:    [16, 2, 64, 112, 112] f32  (value of the window element with the
                                       largest |z|^2 = re^2 + im^2)

Sharding: pure data parallel over batch: 16 / 8 cores = 2 examples per core;
2(batch) x 64(channel) = 128 image planes map 1:1 onto SBUF partitions.

Host layout per row: [even-column (re,im) pairs | odd-column pairs]
([p, h, t, w2, ri]).  One contiguous 25KB-per-partition DMA per 14-row
chunk, and every heavy engine stream (masks, maxes, select data and
pre-fills) is contiguous; only the norm add reads stride-2 and the
copy_predicated masks broadcast step-0 over the (re,im) pair.

  ACT  : squares (one contiguous Square per chunk), select pre-fills
  DVE  : norm add, is_ge masks, horizontal max, copy_predicated selects
  DMA  : 16 input chunks (14 rows), 16 output stores (7 rows), input
         tile triple-buffered so the 9us chunk DMA stays 2 chunks ahead

Selection reproduces jnp.argmax's first-index tie-break (horizontal
is_ge: even/left wins; vertical is_ge: top wins); norm arithmetic is
fl(fl(re^2)+fl(im^2)), bit-exact with the reference.
"""

import numpy as np

import concourse.bass as bass
import concourse.mybir as mybir
from concourse import bacc, bass_utils, tile

NCORES = 8
B = 2            # batch per core
RI = 2           # real/imag
C = 64           # channels
H = W = 224
HO, WO = H // 2, W // 2
P = 128          # SBUF partitions = B * C
R = 14           # image rows per chunk (one DMA = one compute step)
NCHUNK = H // R  # 16
W2 = W // 2      # 112 column pairs
RP = R // 2      # 7 output rows per chunk

F32 = mybir.dt.float32
BF16 = mybir.dt.bfloat16
U8 = mybir.dt.uint8
OP = mybir.AluOpType
ACTF = mybir.ActivationFunctionType

# chunk row schedule: tiny leading chunks collapse the pipeline-fill ramp
# (a full 14-row chunk needs ~9us DMA + 5.5us of squares before the vector
# engine can start); sizes must be even and sum to H
CHUNKS = [2, 4, 8] + [14] * 15
assert sum(CHUNKS) == H

_NC_CACHE = []


def _build_nc() -> bass.Bass:
    nc = bacc.Bacc("TRN2", target_bir_lowering=False, debug=False)
    x = nc.dram_tensor("x", [P, H, W * RI], F32, kind="ExternalInput").ap()
    out = nc.dram_tensor("out", [P, HO, WO * RI], F32, kind="ExternalOutput").ap()

    starts = [sum(CHUNKS[:j]) for j in range(len(CHUNKS))]
    LOOKAHEAD = 3  # input DMAs issued this many chunks ahead of the output
    # DMAs so the in-order trigger queue never stalls the input prefetch

    with tile.TileContext(nc) as tc:
        with tc.tile_pool(name="pool", bufs=2) as pool:
            xT = {}

            def load(j):
                if j >= len(CHUNKS):
                    return
                Rj = CHUNKS[j]
                xri = pool.tile(
                    [P, R * W * RI], F32, tag="xri", name="xri", bufs=LOOKAHEAD
                )
                xT[j] = xri
                nc.sync.dma_start(
                    out=xri[:, : Rj * W * RI].rearrange("p (r f) -> p r f", r=Rj),
                    in_=x[:, starts[j] : starts[j] + Rj, :],
                )

            for j in range(LOOKAHEAD):
                load(j)

            for k, Rk in enumerate(CHUNKS):
                r0 = starts[k]
                RPk = Rk // 2
                NH = Rk * W2             # horizontal windows per chunk
                NO = RPk * W2            # output windows per chunk
                NVAL = Rk * W * RI       # f32 values per chunk

                xri = xT.pop(k)[:, :NVAL]

                # squares of the whole chunk in one contiguous ACT op
                sq = pool.tile([P, R * W * RI], F32, tag="sq", name="sq")[:, :NVAL]
                nc.scalar.activation(out=sq, in_=xri, func=ACTF.Square)

                # norm2 = re^2 + im^2 (stride-2 reads, contiguous out)
                nrm = pool.tile([P, R * W2 * 2], F32, tag="nrm", name="nrm")[:, : NH * 2]
                sqp = sq.rearrange("p (n ri) -> p n ri", ri=RI)
                nc.vector.tensor_tensor(
                    out=nrm, in0=sqp[:, :, 0], in1=sqp[:, :, 1], op=OP.add
                )
                nE3 = nrm.rearrange("p (r t w2) -> p r t w2", r=Rk, t=2, w2=W2)
                nE, nO = nE3[:, :, 0, :], nE3[:, :, 1, :]

                # horizontal mask (even/left wins ties) + horizontal max.
                # masks are bf16 0/1.0: bitcast to u8 gives bytes (0x80,0x3f)
                # per true lane -- a pair-broadcast mask with no step-0 AP
                cH = pool.tile([P, R * W2], BF16, tag="cH", name="cH")[:, :NH]
                cH3 = cH.rearrange("p (r w2) -> p r w2", r=Rk, w2=W2)
                nc.vector.tensor_tensor(out=cH3, in0=nE, in1=nO, op=OP.is_ge)
                mH = pool.tile([P, R * W2], F32, tag="mH", name="mH")[:, :NH]
                mH3 = mH.rearrange("p (r w2) -> p r w2", r=Rk, w2=W2)
                nc.vector.tensor_tensor(out=mH3, in0=nE, in1=nO, op=OP.max)

                # horizontal select of (re,im) pairs: pre-fill odd-column
                # candidates (contiguous block), overwrite where even wins
                xp = xri.rearrange(
                    "p (r t w2 ri) -> p r t w2 ri", r=Rk, t=2, w2=W2, ri=RI
                )
                riH = pool.tile([P, R * W2 * RI], F32, tag="riH", name="riH")[:, : NH * RI]
                riH4 = riH.rearrange(
                    "p (r w2 ri) -> p r w2 ri", r=Rk, w2=W2, ri=RI
                )
                nc.scalar.copy(out=riH4, in_=xp[:, :, 1, :, :])
                nc.vector.copy_predicated(
                    out=riH4,
                    mask=cH.bitcast(U8).rearrange(
                        "p (r w2 ri) -> p r w2 ri", r=Rk, w2=W2, ri=RI
                    ),
                    data=xp[:, :, 0, :, :],
                )

                # vertical mask from horizontal maxes (top wins ties)
                mHr = mH.rearrange(
                    "p (rp rt w2) -> p rp rt w2", rp=RPk, rt=2, w2=W2
                )
                cV = pool.tile([P, RP * W2], BF16, tag="cV", name="cV")[:, :NO]
                cV3 = cV.rearrange("p (rp w2) -> p rp w2", rp=RPk, w2=W2)
                nc.vector.tensor_tensor(
                    out=cV3, in0=mHr[:, :, 0, :], in1=mHr[:, :, 1, :], op=OP.is_ge
                )

                # vertical select into the output tile
                riHr = riH.rearrange(
                    "p (rp rt w2 ri) -> p rp rt w2 ri", rp=RPk, rt=2, w2=W2, ri=RI
                )
                outT = pool.tile([P, RP * W2 * RI], F32, tag="outT", name="outT")[:, : NO * RI]
                outT4 = outT.rearrange(
                    "p (rp w2 ri) -> p rp w2 ri", rp=RPk, w2=W2, ri=RI
                )
                nc.scalar.copy(out=outT4, in_=riHr[:, :, 1, :, :])
                nc.vector.copy_predicated(
                    out=outT4,
                    mask=cV.bitcast(U8).rearrange(
                        "p (rp w2 ri) -> p rp w2 ri", rp=RPk, w2=W2, ri=RI
                    ),
                    data=riHr[:, :, 0, :, :],
                )

                load(k + LOOKAHEAD)
                nc.sync.dma_start(
                    out=out[:, r0 // 2 : r0 // 2 + RPk, :],
                    in_=outT.rearrange("p (rp f) -> p rp f", rp=RPk),
                )
    nc.compile()
    return nc


def get_nc() -> bass.Bass:
    if not _NC_CACHE:
        _NC_CACHE.append(_build_nc())
    return _NC_CACHE[0]


def kernel(x: np.ndarray, **run_kwargs) -> np.ndarray:
    nc = get_nc()
    xs = np.asarray(x, dtype=np.float32)
    assert xs.shape == (NCORES * B, RI, C, H, W), xs.shape
    # [16,2,64,H,W] -> [16,64,H,W2,t,2] -> row blocks [16,64,H,t,W2,2]
    xt = xs.transpose(0, 2, 3, 4, 1).reshape(NCORES * B, C, H, W2, 2, RI)
    xt = np.ascontiguousarray(xt.transpose(0, 1, 2, 4, 3, 5))
    in_maps = [
        {"x": xt[B * i : B * (i + 1)].reshape(P, H, W * RI)} for i in range(NCORES)
    ]
    res = bass_utils.run_bass_kernel_spmd(
        nc, in_maps, core_ids=list(range(NCORES)), **run_kwargs
    )
    # per-core [128, HO, WO*2] -> [b, c, HO, WO, ri] -> [b, ri, c, HO, WO]
    outs = [
        res.results[i]["out"].reshape(B, C, HO, WO, RI).transpose(0, 4, 1, 2, 3)
        for i in range(NCORES)
    ]
    out = np.concatenate(outs, axis=0)
    if run_kwargs:
        kernel.last_results = res
    return np.ascontiguousarray(out)
